# revision 29
# baseline (speedup 1.0000x reference)
"""Multi-head attention (B=2, S=4096, D_MODEL=512, H=8) on 8 TRN2 NeuronCores.

Sharding (data + head/tensor parallel, per the problem's sharding hint):
each core owns (batch b = core//4, head-pair hp = core%4).
 - Q/K/V are batch-sharded (cores sharing b get the same activations,
   pre-transposed to [D, S] on the host so the PE can contract over D).
 - W_q/W_k/W_v are column-sharded per head pair (128 output dims/core).
 - W_o is row-sharded: each core emits a partial [S, D] output and the
   host sums the four partials per batch (the "all-reduce" unshard of a
   row-sharded matmul); b_o is added by one core per batch.
 - Keys with mask==0 contribute nothing to attention, so only the live
   keys are gathered (padded to a multiple of 512; pads carry mask 0 and
   cancel on-device exactly like masked keys would).

Device pipeline per core (all matmuls bf16 with fp32 accumulation):
  1. Staging: K.T/V.T/Q.T land in SBUF via a few large DMAs spread over
     the SP/GpSimd/ACT DGE queues so the three streams overlap.
  2. Projections: qT/kT/vT[d2h, s] = W_x.T-slice.T @ X.T-tiles (N=512
     matmuls), + per-partition bias, cast to bf16.  q/k are stored per
     head zero-padded to 128 partitions so score matmuls contract over
     K=128 (fast weight load + near-perfect LDW/MM pipelining).  v is
     PE-transposed per 128-block into v[k, d2h] tiles stored as
     [k, 65]: column 64 holds the key mask value itself - this both
     applies the mask (v and the ones column are zeroed for dead keys)
     and yields the softmax denominator as row 64 of the MM2 output.
  3. Attention, per (head, 512-wide q-chunk), in k-rounds of 3 tiles:
     S_T[k, q] = kT-tile.T @ qT (transposed scores - no transposes
     needed anywhere in the attention path), exp on ScalarE straight
     from 3 PSUM banks (softmax scale folded into the activation's
     scale field; no max-subtraction needed since |s/8| <= ~6), P ->
     bf16 SBUF, acc[65, q] += v_aug.T @ P accumulated over all k-tiles.
     The next round's score matmuls are emitted ahead of this round's
     MM2s so the PE always has independent work while ScalarE runs exp.
     Normalization divides by row 64 (denominator) broadcast across
     partitions with a rank-1 f32r matmul (gpsimd partition_broadcast
     misreads nonzero base partitions on HW) + fast reciprocal.
  4. out[s, :] = sum_h [attn_T_h|1].T @ [W_o slice.T|b_o] - the stored
     attn rows feed straight in as the stationary operand (row 64 = 1
     adds the bias once).  Emission is deferred a few steps so its
     LDWEIGHTS never stalls the PE on the normalization chain.
"""

import sys

for _p in ("/opt/trn_rl_repo", "/opt/pypackages"):
    if _p not in sys.path:
        sys.path.append(_p)

import numpy as np
import ml_dtypes

B = 2
S = 4096
D = 512
H = 8
DK = 64
N_CORES = 8

P = 128          # partitions
QC = 512         # q-chunk width
N_SC = S // QC   # 8 q-chunks
N_ST = S // P    # 32 s-tiles
N_DT = D // P    # 4 D-tiles (contraction tiles for projections)

_COMPILED = {}
_LAST_IN_MAPS = None
_LAST_RESULTS = None
_LAST_NKT = None

COMPACT = True


def _build(nkt: int):
    """Build + compile the per-core bass program. nkt = number of
    128-wide key tiles (32 = full, smaller when keys are compacted)."""
    import concourse.bass as bass  # noqa: F401
    from concourse.masks import make_identity
    import concourse.mybir as mybir
    import concourse.tile as tile
    from concourse import bacc

    f32 = mybir.dt.float32
    f32r = mybir.dt.float32r
    bf16 = mybir.dt.bfloat16
    EXP = mybir.ActivationFunctionType.Exp

    nk = nkt * P
    n_kc = nk // QC

    nc = bacc.Bacc("TRN2", target_bir_lowering=False, debug=False,
                   enable_asserts=False)

    QT = nc.dram_tensor("QT", [D, S], bf16, kind="ExternalInput").ap()
    KT = nc.dram_tensor("KT", [D, nk], bf16, kind="ExternalInput").ap()
    VT = nc.dram_tensor("VT", [D, nk], bf16, kind="ExternalInput").ap()
    WQT = nc.dram_tensor("WQT", [D, P], bf16, kind="ExternalInput").ap()
    WKT = nc.dram_tensor("WKT", [D, P], bf16, kind="ExternalInput").ap()
    WVT = nc.dram_tensor("WVT", [D, P], bf16, kind="ExternalInput").ap()
    WOB = nc.dram_tensor("WOB", [2, P, D], bf16, kind="ExternalInput").ap()
    BQ = nc.dram_tensor("BQ", [P, 1], f32, kind="ExternalInput").ap()
    BK = nc.dram_tensor("BK", [P, 1], f32, kind="ExternalInput").ap()
    BV = nc.dram_tensor("BV", [P, 1], f32, kind="ExternalInput").ap()
    MASKF = nc.dram_tensor("MASKF", [P, nkt], f32, kind="ExternalInput").ap()
    OUT = nc.dram_tensor("OUT", [S, D], f32, kind="ExternalOutput").ap()

    with tile.TileContext(nc) as tc:
        with tc.tile_pool(name="persist", bufs=1) as persist:
            # ---- persistent SBUF tensors ----
            # per-head [128, *] tensors zero-padded in the unused 64
            # partitions so MM1 can contract over K=128 (enables FWL and
            # near-perfect LDW/MM pipelining)
            qh_sb = [persist.tile([P, S], bf16, name=f"qh{h}_sb")
                     for h in range(2)]
            kh_sb = [persist.tile([P, nk], bf16, name=f"kh{h}_sb")
                     for h in range(2)]
            v0_sb = persist.tile([P, nkt, 65], bf16)  # head0 [k, kt, v|mask]
            v1_sb = persist.tile([P, nkt, 65], bf16)  # head1
            a0_sb = persist.tile([P, S], bf16)      # head0 [attn_T|1|0, q]
            a1_sb = persist.tile([P, S], bf16)      # head1
            # raw input staging (few large DMAs beat many small ones:
            # each dma_start costs ~0.6-1us of DGE sequencer time)
            kin_sb = persist.tile([P, N_DT, nk], bf16)
            vin_sb = persist.tile([P, N_DT, nk], bf16)
            qin_sb = persist.tile([P, N_DT, S], bf16)
            wq_sb = persist.tile([P, N_DT, P], bf16)
            wk_sb = persist.tile([P, N_DT, P], bf16)
            wv_sb = persist.tile([P, N_DT, P], bf16)
            wo0_sb = persist.tile([P, D], bf16)
            wo1_sb = persist.tile([P, D], bf16)
            bq_sb = persist.tile([P, 1], f32)
            bk_sb = persist.tile([P, 1], f32)
            bv_sb = persist.tile([P, 1], f32)
            ident_sb = persist.tile([P, P], bf16)
            ones64_sb = persist.tile([65, DK], f32r)
            ones64_f = persist.tile([65, DK], f32)
            mask_sb = persist.tile([P, nkt], f32)

            for t in (*qh_sb, *kh_sb):
                nc.vector.memset(t, 0.0)
            nc.vector.memset(a0_sb, 0.0)
            nc.vector.memset(a1_sb, 0.0)
            make_identity(nc, ident_sb)
            nc.vector.memset(ones64_f, 1.0)
            nc.vector.tensor_copy(out=ones64_sb, in_=ones64_f)

            # DMA queue assignment: each DGE queue sustains only
            # ~110GB/s, so K and V halves are split across the SP and
            # GpSimd queues (both fully needed before attention starts),
            # while ACT carries the setup tensors + the early Q chunks.
            # Q's tail rides behind K/V on SP/GpSimd, split by need time.
            half = (n_kc // 2) * QC
            for dt in range(N_DT):
                dsl = slice(dt * P, (dt + 1) * P)
                nc.sync.dma_start(out=wk_sb[:, dt, :], in_=WKT[dsl, :])
            for dt in range(N_DT):
                dsl = slice(dt * P, (dt + 1) * P)
                nc.sync.dma_start(out=kin_sb[:, dt, 0:half],
                                  in_=KT[dsl, 0:half])
                nc.gpsimd.dma_start(out=vin_sb[:, dt, 0:half],
                                    in_=VT[dsl, 0:half])
            for dt in range(N_DT):
                dsl = slice(dt * P, (dt + 1) * P)
                nc.gpsimd.dma_start(out=kin_sb[:, dt, half:nk],
                                    in_=KT[dsl, half:nk])
                nc.sync.dma_start(out=vin_sb[:, dt, half:nk],
                                  in_=VT[dsl, half:nk])
            for dt in range(N_DT):
                dsl = slice(dt * P, (dt + 1) * P)
                nc.scalar.dma_start(out=wv_sb[:, dt, :], in_=WVT[dsl, :])
                nc.scalar.dma_start(out=wq_sb[:, dt, :], in_=WQT[dsl, :])
            nc.scalar.dma_start(out=bk_sb, in_=BK)
            nc.scalar.dma_start(out=bv_sb, in_=BV)
            nc.scalar.dma_start(out=bq_sb, in_=BQ)
            nc.scalar.dma_start(out=mask_sb, in_=MASKF)
            for dt in range(N_DT):
                dsl = slice(dt * P, (dt + 1) * P)
                nc.scalar.dma_start(out=qin_sb[:, dt, 0:QC],
                                    in_=QT[dsl, 0:QC])
            nc.scalar.dma_start(out=wo0_sb, in_=WOB[0])
            nc.scalar.dma_start(out=wo1_sb, in_=WOB[1])
            # Q chunks 1-3 on ACT (needed first), 4-7 behind K/V
            for dt in range(N_DT):
                dsl = slice(dt * P, (dt + 1) * P)
                nc.scalar.dma_start(out=qin_sb[:, dt, QC:4 * QC],
                                    in_=QT[dsl, QC:4 * QC])
            for dt in range(N_DT):
                dsl = slice(dt * P, (dt + 1) * P)
                eng = nc.sync if dt < 2 else nc.gpsimd
                eng.dma_start(out=qin_sb[:, dt, 4 * QC:S],
                              in_=QT[dsl, 4 * QC:S])
            # column 64 of every v tile = mask value (denominator source)
            nc.vector.tensor_copy(out=v0_sb[:, :, 64], in_=mask_sb)
            nc.vector.tensor_copy(out=v1_sb[:, :, 64], in_=mask_sb)

            # ================= phase P: projections =================
            pin = tc.alloc_tile_pool(name="pin", bufs=2)
            with tc.tile_pool(name="pps", bufs=2, space="PSUM") as pps:

                def qkproj(xin_sb, w_sb, b_sb, x_out, sc, pool=None,
                           tag="qk"):
                    ssl = slice(sc * QC, (sc + 1) * QC)
                    ps = (pool or pps).tile([P, QC], f32, tag=tag,
                                            name="ps", bufs=2)
                    for dt in range(N_DT):
                        nc.tensor.matmul(ps, lhsT=w_sb[:, dt, :],
                                         rhs=xin_sb[:, dt, ssl],
                                         start=(dt == 0),
                                         stop=(dt == N_DT - 1))
                    nc.vector.tensor_scalar_add(
                        out=x_out[0][0:DK, ssl], in0=ps[0:DK, :],
                        scalar1=b_sb[0:DK, :])
                    nc.vector.tensor_scalar_add(
                        out=x_out[1][DK:P, ssl], in0=ps[DK:P, :],
                        scalar1=b_sb[DK:P, :])

                def vproj(vc):
                    # vT[d2h, chunk] (N=512 matmuls), then one PE
                    # transpose per 128-block gives v[k, d2h] for both
                    # heads at once
                    ssl = slice(vc * QC, (vc + 1) * QC)
                    ps = pps.tile([P, QC], f32, tag="qk", name="vps",
                                  bufs=2)
                    for dt in range(N_DT):
                        nc.tensor.matmul(ps, lhsT=wv_sb[:, dt, :],
                                         rhs=vin_sb[:, dt, ssl],
                                         start=(dt == 0),
                                         stop=(dt == N_DT - 1))
                    vt_sb = pin.tile([P, QC], bf16, tag="vt", bufs=2,
                                     name="vt_sb")
                    nc.vector.tensor_scalar_add(out=vt_sb, in0=ps,
                                                scalar1=bv_sb)
                    for st4 in range(QC // P):
                        st = vc * (QC // P) + st4
                        tp = pps.tile([P, P], bf16, tag="v", name="tp",
                                      bufs=2)
                        nc.tensor.transpose(
                            tp, vt_sb[:, st4 * P:(st4 + 1) * P], ident_sb)
                        msl = mask_sb[:, st:st + 1]
                        nc.vector.tensor_scalar_mul(
                            out=v0_sb[:, st, 0:DK], in0=tp[:, 0:DK],
                            scalar1=msl)
                        nc.vector.tensor_scalar_mul(
                            out=v1_sb[:, st, 0:DK], in0=tp[:, DK:P],
                            scalar1=msl)

                for c in range(n_kc):
                    qkproj(kin_sb, wk_sb, bk_sb, kh_sb, c)
                    vproj(c)
                qkproj(qin_sb, wq_sb, bq_sb, qh_sb, 0)
                _qkproj = qkproj

            # ================= phase A: attention =================
            # k-tile rounds of <=3 (exp reads 3 PSUM banks at once)
            rounds = []
            ki = 0
            while ki < nkt:
                n = min(3, nkt - ki)
                rounds.append((ki, n))
                ki += n

            with tc.tile_pool(name="aps", bufs=1, space="PSUM") as aps, \
                 tc.tile_pool(name="asb", bufs=1) as asb:
                v_sbs = (v0_sb, v1_sb)
                a_sbs = (a0_sb, a1_sb)
                steps = [(h, sc, k0, nk_r)
                         for h in (0, 1)
                         for sc in range(N_SC)
                         for (k0, nk_r) in rounds]

                def emit_mm1(step):
                    h, sc, k0, nk_r = step
                    s_ps = aps.tile([P, 3 * QC], f32, tag="mm1", bufs=2,
                                    name="s_ps")
                    for j in range(nk_r):
                        kt = k0 + j
                        nc.tensor.matmul(
                            s_ps[:, j * QC:(j + 1) * QC],
                            lhsT=kh_sb[h][:, kt * P:(kt + 1) * P],
                            rhs=qh_sb[h][:, sc * QC:(sc + 1) * QC],
                            start=True, stop=True)
                    return s_ps

                def emit_outproj(sc):
                    for st4 in range(QC // P):
                        st = sc * (QC // P) + st4
                        tsl = slice(st * P, (st + 1) * P)
                        po = aps.tile([P, D], f32, tag="acc", bufs=2,
                                      name="po")
                        nc.tensor.matmul(po, lhsT=a0_sb[:, tsl],
                                         rhs=wo0_sb, start=True, stop=False)
                        nc.tensor.matmul(po, lhsT=a1_sb[:, tsl],
                                         rhs=wo1_sb, start=False, stop=True)
                        osb = asb.tile([P, D], f32, tag="osb", bufs=3)
                        nc.vector.tensor_copy(out=osb, in_=po)
                        nc.sync.dma_start(out=OUT[tsl, :], in_=osb)

                s_ps_next = emit_mm1(steps[0])
                acc = None
                pending = []  # (sc, step index when ready)
                for i, step in enumerate(steps):
                    h, sc, k0, nk_r = step
                    ssl = slice(sc * QC, (sc + 1) * QC)
                    v_sb, a_sb = v_sbs[h], a_sbs[h]
                    if h == 0 and k0 == 0 and sc + 1 < N_SC:
                        # project the next q-chunk while this one runs
                        _qkproj(qin_sb, wq_sb, bq_sb, qh_sb, sc + 1,
                                pool=aps, tag="acc")
                    s_ps = s_ps_next
                    p_sb = asb.tile([P, 3 * QC], bf16, tag="p", bufs=3)
                    nc.scalar.activation(
                        p_sb[:, :nk_r * QC], s_ps[:, :nk_r * QC],
                        EXP, bias=0.0, scale=0.125)
                    if i + 1 < len(steps):
                        s_ps_next = emit_mm1(steps[i + 1])
                    if k0 == 0:
                        acc = aps.tile([P, QC], f32, tag="acc", bufs=2,
                                       name="acc")
                    for j in range(nk_r):
                        kt = k0 + j
                        nc.tensor.matmul(
                            acc[0:65, :],
                            lhsT=v_sb[:, kt, :],
                            rhs=p_sb[:, j * QC:(j + 1) * QC],
                            start=(kt == 0), stop=(kt == nkt - 1))
                    if k0 + nk_r == nkt:
                        # normalize: attn = acc[0:64] / acc[64].  Broadcast
                        # the denominator row across partitions with a
                        # rank-1 matmul (f32r streams at full rate; gpsimd
                        # partition_broadcast misreads nonzero base
                        # partitions on HW), then fast reciprocal at base
                        # partition 0.
                        den = asb.tile([65, QC], f32r, tag="den", bufs=2)
                        nc.vector.tensor_copy(out=den[64:65, :],
                                              in_=acc[64:65, :])
                        rb_ps = aps.tile([P, QC], f32, tag="acc", bufs=2,
                                         name="rb_ps")
                        nc.tensor.matmul(
                            rb_ps[0:64, :],
                            lhsT=ones64_sb[64:65, :],
                            rhs=den[64:65, :],
                            start=True, stop=True)
                        recb = asb.tile([64, QC], f32, tag="recb", bufs=2)
                        nc.vector.reciprocal_approx_fast(out=recb,
                                                         in_=rb_ps[0:64, :])
                        nc.vector.tensor_mul(
                            out=a_sb[0:64, ssl], in0=acc[0:64, :],
                            in1=recb)
                        nc.vector.memset(a_sb[64:65, ssl], 1.0)
                        if h == 1:
                            pending.append((sc, i))
                    while pending and i - pending[0][1] >= 4:
                        emit_outproj(pending.pop(0)[0])
                for sc, _ in pending:
                    emit_outproj(sc)
            pin.release()

    nc.compile()
    return nc


def _get_compiled(nkt: int):
    if nkt not in _COMPILED:
        _COMPILED[nkt] = _build(nkt)
    return _COMPILED[nkt]


def kernel(Q, K, V, mask, W_q, b_q, W_k, b_k, W_v, b_v, W_o, b_o):
    from concourse import bass_utils

    bf16 = ml_dtypes.bfloat16
    mask = np.asarray(mask)
    if COMPACT:
        # keys with mask==0 contribute nothing to attention: gather only
        # the live keys (padded to a multiple of 512; pads carry mask 0
        # and are cancelled on-device exactly like masked keys)
        idxs = [np.flatnonzero(mask[b]) for b in range(B)]
        nkt = max(1, -(-max(len(ix) for ix in idxs) // P))
        nkt = min(-(-nkt // 4) * 4, N_ST)  # multiple of 4: 512-wide chunks
    else:
        idxs = None
        nkt = N_ST
    nk = nkt * P

    Q = np.asarray(Q, np.float32)
    K = np.asarray(K, np.float32)
    V = np.asarray(V, np.float32)
    W_q = np.asarray(W_q, np.float32)
    W_k = np.asarray(W_k, np.float32)
    W_v = np.asarray(W_v, np.float32)
    W_o = np.asarray(W_o, np.float32)
    b_q = np.asarray(b_q, np.float32)
    b_k = np.asarray(b_k, np.float32)
    b_v = np.asarray(b_v, np.float32)
    b_o = np.asarray(b_o, np.float32)

    nc = _get_compiled(nkt)

    in_maps = []
    for core in range(N_CORES):
        b = core // 4
        hp = core % 4
        rsl = slice(hp * P, (hp + 1) * P)   # rows of W_q/W_k/W_v, 2 heads
        h0, h1 = 2 * hp, 2 * hp + 1

        qt = np.ascontiguousarray(Q[b].T).astype(bf16)
        if COMPACT:
            ix = idxs[b]
            nkeep = len(ix)
            ixp = np.zeros(nk, np.int64)
            ixp[:nkeep] = ix[:nk]
            kt = np.ascontiguousarray(K[b][ixp, :].T).astype(bf16)
            vt = np.ascontiguousarray(V[b][ixp, :].T).astype(bf16)
            mvec = (np.arange(nk) < min(nkeep, nk)).astype(np.float32)
        else:
            kt = np.ascontiguousarray(K[b].T).astype(bf16)
            vt = np.ascontiguousarray(V[b].T).astype(bf16)
            mvec = mask[b].astype(np.float32)

        wob = np.zeros((2, P, D), np.float32)
        if hp == 0:
            wob[0, 64, :] = b_o
        wob[0, 0:64, :] = W_o[:, h0 * DK:(h0 + 1) * DK].T
        wob[1, 0:64, :] = W_o[:, h1 * DK:(h1 + 1) * DK].T

        maskf = np.ascontiguousarray(mvec.reshape(nkt, P).T)

        in_maps.append({
            "QT": qt,
            "KT": kt,
            "VT": vt,
            "WQT": np.ascontiguousarray(W_q[rsl, :].T).astype(bf16),
            "WKT": np.ascontiguousarray(W_k[rsl, :].T).astype(bf16),
            "WVT": np.ascontiguousarray(W_v[rsl, :].T).astype(bf16),
            "WOB": wob.astype(bf16),
            "BQ": np.ascontiguousarray(b_q[rsl].reshape(P, 1)),
            "BK": np.ascontiguousarray(b_k[rsl].reshape(P, 1)),
            "BV": np.ascontiguousarray(b_v[rsl].reshape(P, 1)),
            "MASKF": maskf,
        })

    global _LAST_IN_MAPS, _LAST_RESULTS, _LAST_NKT
    _LAST_IN_MAPS = in_maps
    _LAST_NKT = nkt

    res = bass_utils.run_bass_kernel_spmd(
        nc, in_maps, core_ids=list(range(N_CORES)))

    _LAST_RESULTS = res.results

    out = np.zeros((B, S, D), np.float32)
    for core in range(N_CORES):
        out[core // 4] += np.asarray(res.results[core]["OUT"], np.float32)
    return out


# revision 30
# speedup vs baseline: 1.1748x; 1.1748x over previous
"""Multi-head attention (B=2, S=4096, D_MODEL=512, H=8) on 8 TRN2 NeuronCores.

Sharding (data + head/tensor parallel, per the problem's sharding hint):
each core owns (batch b = core//4, head-pair hp = core%4).
 - Q/K/V are batch-sharded (cores sharing b get the same activations,
   pre-transposed to [D, S] on the host so the PE can contract over D).
 - W_q/W_k/W_v are column-sharded per head pair (128 output dims/core).
 - W_o is row-sharded: each core emits a partial [S, D] output and the
   host sums the four partials per batch (the "all-reduce" unshard of a
   row-sharded matmul); b_o is added by one core per batch.
 - Keys with mask==0 contribute nothing to attention, so only the live
   keys are gathered (padded to a multiple of 512; pads carry mask 0 and
   cancel on-device exactly like masked keys would).

Device pipeline per core (all matmuls bf16 with fp32 accumulation):
  1. Staging: K.T/V.T/Q.T land in SBUF via a few large DMAs spread over
     the SP/GpSimd/ACT DGE queues so the three streams overlap.
  2. Projections: qT/kT/vT[d2h, s] = W_x.T-slice.T @ X.T-tiles (N=512
     matmuls), + per-partition bias, cast to bf16.  q/k are stored per
     head zero-padded to 128 partitions so score matmuls contract over
     K=128 (fast weight load + near-perfect LDW/MM pipelining).  v is
     PE-transposed per 128-block into v[k, d2h] tiles stored as
     [k, 65]: column 64 holds the key mask value itself - this both
     applies the mask (v and the ones column are zeroed for dead keys)
     and yields the softmax denominator as row 64 of the MM2 output.
  3. Attention, per (head, 512-wide q-chunk), in k-rounds of 3 tiles:
     S_T[k, q] = kT-tile.T @ qT (transposed scores - no transposes
     needed anywhere in the attention path), exp on ScalarE straight
     from 3 PSUM banks (softmax scale folded into the activation's
     scale field; no max-subtraction needed since |s/8| <= ~6), P ->
     bf16 SBUF, acc[65, q] += v_aug.T @ P accumulated over all k-tiles.
     The next round's score matmuls are emitted ahead of this round's
     MM2s so the PE always has independent work while ScalarE runs exp.
     Normalization divides by row 64 (denominator) broadcast across
     partitions with a rank-1 f32r matmul (gpsimd partition_broadcast
     misreads nonzero base partitions on HW) + fast reciprocal.
  4. out[s, :] = sum_h [attn_T_h|1].T @ [W_o slice.T|b_o] - the stored
     attn rows feed straight in as the stationary operand (row 64 = 1
     adds the bias once).  Emission is deferred a few steps so its
     LDWEIGHTS never stalls the PE on the normalization chain.
"""

import sys

for _p in ("/opt/trn_rl_repo", "/opt/pypackages"):
    if _p not in sys.path:
        sys.path.append(_p)

import numpy as np
import ml_dtypes

B = 2
S = 4096
D = 512
H = 8
DK = 64
N_CORES = 8

P = 128          # partitions
QC = 512         # q-chunk width
N_SC = S // QC   # 8 q-chunks
N_ST = S // P    # 32 s-tiles
N_DT = D // P    # 4 D-tiles (contraction tiles for projections)

_COMPILED = {}
_LAST_IN_MAPS = None
_LAST_RESULTS = None
_LAST_NKT = None

COMPACT = True


def _build(nkt: int):
    """Build + compile the per-core bass program. nkt = number of
    128-wide key tiles (32 = full, smaller when keys are compacted)."""
    import concourse.bass as bass  # noqa: F401
    from concourse.masks import make_identity
    import concourse.mybir as mybir
    import concourse.tile as tile
    from concourse import bacc

    f32 = mybir.dt.float32
    f32r = mybir.dt.float32r
    bf16 = mybir.dt.bfloat16
    EXP = mybir.ActivationFunctionType.Exp

    nk = nkt * P
    n_kc = nk // QC

    nc = bacc.Bacc("TRN2", target_bir_lowering=False, debug=False,
                   enable_asserts=False)

    QT = nc.dram_tensor("QT", [D, S], bf16, kind="ExternalInput").ap()
    KT = nc.dram_tensor("KT", [D, nk], bf16, kind="ExternalInput").ap()
    VT = nc.dram_tensor("VT", [D, nk], bf16, kind="ExternalInput").ap()
    WQT = nc.dram_tensor("WQT", [D, P], bf16, kind="ExternalInput").ap()
    WKT = nc.dram_tensor("WKT", [D, P], bf16, kind="ExternalInput").ap()
    WVT = nc.dram_tensor("WVT", [D, P], bf16, kind="ExternalInput").ap()
    WOB = nc.dram_tensor("WOB", [2, P, D], bf16, kind="ExternalInput").ap()
    BQ = nc.dram_tensor("BQ", [P, 1], f32, kind="ExternalInput").ap()
    BK = nc.dram_tensor("BK", [P, 1], f32, kind="ExternalInput").ap()
    BV = nc.dram_tensor("BV", [P, 1], f32, kind="ExternalInput").ap()
    MASKF = nc.dram_tensor("MASKF", [P, nkt], f32, kind="ExternalInput").ap()
    OUT = nc.dram_tensor("OUT", [S, D], f32, kind="ExternalOutput").ap()

    with tile.TileContext(nc) as tc:
        with tc.tile_pool(name="persist", bufs=1) as persist:
            # ---- persistent SBUF tensors ----
            # per-head [128, *] tensors zero-padded in the unused 64
            # partitions so MM1 can contract over K=128 (enables FWL and
            # near-perfect LDW/MM pipelining)
            qh_sb = [persist.tile([P, S], bf16, name=f"qh{h}_sb")
                     for h in range(2)]
            kh_sb = [persist.tile([P, nk], bf16, name=f"kh{h}_sb")
                     for h in range(2)]
            v0_sb = persist.tile([P, nkt, 65], bf16)  # head0 [k, kt, v|mask]
            v1_sb = persist.tile([P, nkt, 65], bf16)  # head1
            a0_sb = persist.tile([P, S], bf16)      # head0 [attn_T|1|0, q]
            a1_sb = persist.tile([P, S], bf16)      # head1
            # raw input staging (few large DMAs beat many small ones:
            # each dma_start costs ~0.6-1us of DGE sequencer time)
            kin_sb = persist.tile([P, N_DT, nk], bf16)
            vin_sb = persist.tile([P, N_DT, nk], bf16)
            qin_sb = persist.tile([P, N_DT, S], bf16)
            wq_sb = persist.tile([P, N_DT, P], bf16)
            wk_sb = persist.tile([P, N_DT, P], bf16)
            wv_sb = persist.tile([P, N_DT, P], bf16)
            wo0_sb = persist.tile([P, D], bf16)
            wo1_sb = persist.tile([P, D], bf16)
            bq_sb = persist.tile([P, 1], f32)
            bk_sb = persist.tile([P, 1], f32)
            bv_sb = persist.tile([P, 1], f32)
            ident_sb = persist.tile([P, P], bf16)
            ones64_sb = persist.tile([65, DK], f32r)
            ones64_f = persist.tile([65, DK], f32)
            mask_sb = persist.tile([P, nkt], f32)

            for t in (*qh_sb, *kh_sb):
                nc.vector.memset(t, 0.0)
            nc.vector.memset(a0_sb, 0.0)
            nc.vector.memset(a1_sb, 0.0)
            make_identity(nc, ident_sb)
            nc.vector.memset(ones64_f, 1.0)
            nc.vector.tensor_copy(out=ones64_sb, in_=ones64_f)

            # DMA queue assignment: SP carries wk + K (the first thing
            # attention needs), GpSimd carries V in parallel, ACT carries
            # the small setup tensors + Q (whose first chunk is split off
            # so q-chunk 0 is projectable within a few us).
            half = (n_kc // 2) * QC
            for dt in range(N_DT):
                dsl = slice(dt * P, (dt + 1) * P)
                nc.sync.dma_start(out=wk_sb[:, dt, :], in_=WKT[dsl, :])
            for dt in range(N_DT):
                dsl = slice(dt * P, (dt + 1) * P)
                nc.sync.dma_start(out=kin_sb[:, dt, 0:half],
                                  in_=KT[dsl, 0:half])
            for dt in range(N_DT):
                dsl = slice(dt * P, (dt + 1) * P)
                nc.sync.dma_start(out=kin_sb[:, dt, half:nk],
                                  in_=KT[dsl, half:nk])
            for dt in range(N_DT):
                dsl = slice(dt * P, (dt + 1) * P)
                nc.gpsimd.dma_start(out=vin_sb[:, dt, 0:half],
                                    in_=VT[dsl, 0:half])
            for dt in range(N_DT):
                dsl = slice(dt * P, (dt + 1) * P)
                nc.gpsimd.dma_start(out=vin_sb[:, dt, half:nk],
                                    in_=VT[dsl, half:nk])
            for dt in range(N_DT):
                dsl = slice(dt * P, (dt + 1) * P)
                nc.scalar.dma_start(out=wv_sb[:, dt, :], in_=WVT[dsl, :])
                nc.scalar.dma_start(out=wq_sb[:, dt, :], in_=WQT[dsl, :])
            nc.scalar.dma_start(out=bk_sb, in_=BK)
            nc.scalar.dma_start(out=bv_sb, in_=BV)
            nc.scalar.dma_start(out=bq_sb, in_=BQ)
            nc.scalar.dma_start(out=mask_sb, in_=MASKF)
            for dt in range(N_DT):
                dsl = slice(dt * P, (dt + 1) * P)
                nc.scalar.dma_start(out=qin_sb[:, dt, 0:QC],
                                    in_=QT[dsl, 0:QC])
            nc.scalar.dma_start(out=wo0_sb, in_=WOB[0])
            nc.scalar.dma_start(out=wo1_sb, in_=WOB[1])
            for dt in range(N_DT):
                dsl = slice(dt * P, (dt + 1) * P)
                nc.scalar.dma_start(out=qin_sb[:, dt, QC:S],
                                    in_=QT[dsl, QC:S])
            # column 64 of every v tile = mask value (denominator source)
            nc.vector.tensor_copy(out=v0_sb[:, :, 64], in_=mask_sb)
            nc.vector.tensor_copy(out=v1_sb[:, :, 64], in_=mask_sb)

            # ================= phase P: projections =================
            pin = tc.alloc_tile_pool(name="pin", bufs=2)
            with tc.tile_pool(name="pps", bufs=2, space="PSUM") as pps:

                def qkproj(xin_sb, w_sb, b_sb, x_out, sc, pool=None,
                           tag="qk"):
                    ssl = slice(sc * QC, (sc + 1) * QC)
                    ps = (pool or pps).tile([P, QC], f32, tag=tag,
                                            name="ps", bufs=2)
                    for dt in range(N_DT):
                        nc.tensor.matmul(ps, lhsT=w_sb[:, dt, :],
                                         rhs=xin_sb[:, dt, ssl],
                                         start=(dt == 0),
                                         stop=(dt == N_DT - 1))
                    nc.vector.tensor_scalar_add(
                        out=x_out[0][0:DK, ssl], in0=ps[0:DK, :],
                        scalar1=b_sb[0:DK, :])
                    nc.vector.tensor_scalar_add(
                        out=x_out[1][DK:P, ssl], in0=ps[DK:P, :],
                        scalar1=b_sb[DK:P, :])

                def vproj(vc):
                    # vT[d2h, chunk] (N=512 matmuls), then one PE
                    # transpose per 128-block gives v[k, d2h] for both
                    # heads at once
                    ssl = slice(vc * QC, (vc + 1) * QC)
                    ps = pps.tile([P, QC], f32, tag="qk", name="vps",
                                  bufs=2)
                    for dt in range(N_DT):
                        nc.tensor.matmul(ps, lhsT=wv_sb[:, dt, :],
                                         rhs=vin_sb[:, dt, ssl],
                                         start=(dt == 0),
                                         stop=(dt == N_DT - 1))
                    vt_sb = pin.tile([P, QC], bf16, tag="vt", bufs=2,
                                     name="vt_sb")
                    nc.vector.tensor_scalar_add(out=vt_sb, in0=ps,
                                                scalar1=bv_sb)
                    for st4 in range(QC // P):
                        st = vc * (QC // P) + st4
                        tp = pps.tile([P, P], bf16, tag="v", name="tp",
                                      bufs=2)
                        nc.tensor.transpose(
                            tp, vt_sb[:, st4 * P:(st4 + 1) * P], ident_sb)
                        msl = mask_sb[:, st:st + 1]
                        nc.vector.tensor_scalar_mul(
                            out=v0_sb[:, st, 0:DK], in0=tp[:, 0:DK],
                            scalar1=msl)
                        nc.vector.tensor_scalar_mul(
                            out=v1_sb[:, st, 0:DK], in0=tp[:, DK:P],
                            scalar1=msl)

                for sc in range(n_kc):
                    qkproj(kin_sb, wk_sb, bk_sb, kh_sb, sc)
                for vc in range(n_kc):
                    vproj(vc)
                qkproj(qin_sb, wq_sb, bq_sb, qh_sb, 0)
                _qkproj = qkproj

            # ================= phase A: attention =================
            # k-tile rounds of <=3 (exp reads 3 PSUM banks at once)
            rounds = []
            ki = 0
            while ki < nkt:
                n = min(3, nkt - ki)
                rounds.append((ki, n))
                ki += n

            with tc.tile_pool(name="aps", bufs=1, space="PSUM") as aps, \
                 tc.tile_pool(name="asb", bufs=1) as asb:
                v_sbs = (v0_sb, v1_sb)
                a_sbs = (a0_sb, a1_sb)
                steps = [(h, sc, k0, nk_r)
                         for h in (0, 1)
                         for sc in range(N_SC)
                         for (k0, nk_r) in rounds]

                def emit_mm1(step):
                    h, sc, k0, nk_r = step
                    s_ps = aps.tile([P, 3 * QC], f32, tag="mm1", bufs=2,
                                    name="s_ps")
                    for j in range(nk_r):
                        kt = k0 + j
                        nc.tensor.matmul(
                            s_ps[:, j * QC:(j + 1) * QC],
                            lhsT=kh_sb[h][:, kt * P:(kt + 1) * P],
                            rhs=qh_sb[h][:, sc * QC:(sc + 1) * QC],
                            start=True, stop=True)
                    return s_ps

                def emit_outproj(sc):
                    for st4 in range(QC // P):
                        st = sc * (QC // P) + st4
                        tsl = slice(st * P, (st + 1) * P)
                        po = aps.tile([P, D], f32, tag="acc", bufs=2,
                                      name="po")
                        nc.tensor.matmul(po, lhsT=a0_sb[:, tsl],
                                         rhs=wo0_sb, start=True, stop=False)
                        nc.tensor.matmul(po, lhsT=a1_sb[:, tsl],
                                         rhs=wo1_sb, start=False, stop=True)
                        osb = asb.tile([P, D], f32, tag="osb", bufs=3)
                        nc.vector.tensor_copy(out=osb, in_=po)
                        nc.sync.dma_start(out=OUT[tsl, :], in_=osb)

                s_ps_next = emit_mm1(steps[0])
                acc = None
                pending = []  # (sc, step index when ready)
                for i, step in enumerate(steps):
                    h, sc, k0, nk_r = step
                    ssl = slice(sc * QC, (sc + 1) * QC)
                    v_sb, a_sb = v_sbs[h], a_sbs[h]
                    if h == 0 and k0 == 0 and sc + 1 < N_SC:
                        # project the next q-chunk while this one runs
                        _qkproj(qin_sb, wq_sb, bq_sb, qh_sb, sc + 1,
                                pool=aps, tag="acc")
                    s_ps = s_ps_next
                    p_sb = asb.tile([P, 3 * QC], bf16, tag="p", bufs=3)
                    nc.scalar.activation(
                        p_sb[:, :nk_r * QC], s_ps[:, :nk_r * QC],
                        EXP, bias=0.0, scale=0.125)
                    if i + 1 < len(steps):
                        s_ps_next = emit_mm1(steps[i + 1])
                    if k0 == 0:
                        acc = aps.tile([P, QC], f32, tag="acc", bufs=2,
                                       name="acc")
                    for j in range(nk_r):
                        kt = k0 + j
                        nc.tensor.matmul(
                            acc[0:65, :],
                            lhsT=v_sb[:, kt, :],
                            rhs=p_sb[:, j * QC:(j + 1) * QC],
                            start=(kt == 0), stop=(kt == nkt - 1))
                    if k0 + nk_r == nkt:
                        # normalize: attn = acc[0:64] / acc[64].  Broadcast
                        # the denominator row across partitions with a
                        # rank-1 matmul (f32r streams at full rate; gpsimd
                        # partition_broadcast misreads nonzero base
                        # partitions on HW), then fast reciprocal at base
                        # partition 0.
                        den = asb.tile([65, QC], f32r, tag="den", bufs=2)
                        nc.vector.tensor_copy(out=den[64:65, :],
                                              in_=acc[64:65, :])
                        rb_ps = aps.tile([P, QC], f32, tag="acc", bufs=2,
                                         name="rb_ps")
                        nc.tensor.matmul(
                            rb_ps[0:64, :],
                            lhsT=ones64_sb[64:65, :],
                            rhs=den[64:65, :],
                            start=True, stop=True)
                        recb = asb.tile([64, QC], f32, tag="recb", bufs=2)
                        nc.vector.reciprocal_approx_fast(out=recb,
                                                         in_=rb_ps[0:64, :])
                        nc.vector.tensor_mul(
                            out=a_sb[0:64, ssl], in0=acc[0:64, :],
                            in1=recb)
                        nc.vector.memset(a_sb[64:65, ssl], 1.0)
                        if h == 1:
                            pending.append((sc, i))
                    while pending and i - pending[0][1] >= 4:
                        emit_outproj(pending.pop(0)[0])
                for sc, _ in pending:
                    emit_outproj(sc)
            pin.release()

    nc.compile()
    return nc


def _get_compiled(nkt: int):
    if nkt not in _COMPILED:
        _COMPILED[nkt] = _build(nkt)
    return _COMPILED[nkt]


def kernel(Q, K, V, mask, W_q, b_q, W_k, b_k, W_v, b_v, W_o, b_o):
    from concourse import bass_utils

    bf16 = ml_dtypes.bfloat16
    mask = np.asarray(mask)
    if COMPACT:
        # keys with mask==0 contribute nothing to attention: gather only
        # the live keys (padded to a multiple of 512; pads carry mask 0
        # and are cancelled on-device exactly like masked keys)
        idxs = [np.flatnonzero(mask[b]) for b in range(B)]
        nkt = max(1, -(-max(len(ix) for ix in idxs) // P))
        nkt = min(-(-nkt // 4) * 4, N_ST)  # multiple of 4: 512-wide chunks
    else:
        idxs = None
        nkt = N_ST
    nk = nkt * P

    Q = np.asarray(Q, np.float32)
    K = np.asarray(K, np.float32)
    V = np.asarray(V, np.float32)
    W_q = np.asarray(W_q, np.float32)
    W_k = np.asarray(W_k, np.float32)
    W_v = np.asarray(W_v, np.float32)
    W_o = np.asarray(W_o, np.float32)
    b_q = np.asarray(b_q, np.float32)
    b_k = np.asarray(b_k, np.float32)
    b_v = np.asarray(b_v, np.float32)
    b_o = np.asarray(b_o, np.float32)

    nc = _get_compiled(nkt)

    in_maps = []
    for core in range(N_CORES):
        b = core // 4
        hp = core % 4
        rsl = slice(hp * P, (hp + 1) * P)   # rows of W_q/W_k/W_v, 2 heads
        h0, h1 = 2 * hp, 2 * hp + 1

        qt = np.ascontiguousarray(Q[b].T).astype(bf16)
        if COMPACT:
            ix = idxs[b]
            nkeep = len(ix)
            ixp = np.zeros(nk, np.int64)
            ixp[:nkeep] = ix[:nk]
            kt = np.ascontiguousarray(K[b][ixp, :].T).astype(bf16)
            vt = np.ascontiguousarray(V[b][ixp, :].T).astype(bf16)
            mvec = (np.arange(nk) < min(nkeep, nk)).astype(np.float32)
        else:
            kt = np.ascontiguousarray(K[b].T).astype(bf16)
            vt = np.ascontiguousarray(V[b].T).astype(bf16)
            mvec = mask[b].astype(np.float32)

        wob = np.zeros((2, P, D), np.float32)
        if hp == 0:
            wob[0, 64, :] = b_o
        wob[0, 0:64, :] = W_o[:, h0 * DK:(h0 + 1) * DK].T
        wob[1, 0:64, :] = W_o[:, h1 * DK:(h1 + 1) * DK].T

        maskf = np.ascontiguousarray(mvec.reshape(nkt, P).T)

        in_maps.append({
            "QT": qt,
            "KT": kt,
            "VT": vt,
            "WQT": np.ascontiguousarray(W_q[rsl, :].T).astype(bf16),
            "WKT": np.ascontiguousarray(W_k[rsl, :].T).astype(bf16),
            "WVT": np.ascontiguousarray(W_v[rsl, :].T).astype(bf16),
            "WOB": wob.astype(bf16),
            "BQ": np.ascontiguousarray(b_q[rsl].reshape(P, 1)),
            "BK": np.ascontiguousarray(b_k[rsl].reshape(P, 1)),
            "BV": np.ascontiguousarray(b_v[rsl].reshape(P, 1)),
            "MASKF": maskf,
        })

    global _LAST_IN_MAPS, _LAST_RESULTS, _LAST_NKT
    _LAST_IN_MAPS = in_maps
    _LAST_NKT = nkt

    res = bass_utils.run_bass_kernel_spmd(
        nc, in_maps, core_ids=list(range(N_CORES)))

    _LAST_RESULTS = res.results

    out = np.zeros((B, S, D), np.float32)
    for core in range(N_CORES):
        out[core // 4] += np.asarray(res.results[core]["OUT"], np.float32)
    return out


# revision 31
# speedup vs baseline: 1.1779x; 1.0026x over previous
"""Multi-head attention (B=2, S=4096, D_MODEL=512, H=8) on 8 TRN2 NeuronCores.

Sharding (data + head/tensor parallel, per the problem's sharding hint):
each core owns (batch b = core//4, head-pair hp = core%4).
 - Q/K/V are batch-sharded (cores sharing b get the same activations,
   pre-transposed to [D, S] on the host so the PE can contract over D).
 - W_q/W_k/W_v are column-sharded per head pair (128 output dims/core).
 - W_o is row-sharded: each core emits a partial [S, D] output and the
   host sums the four partials per batch (the "all-reduce" unshard of a
   row-sharded matmul); b_o is added by one core per batch.
 - Keys with mask==0 contribute nothing to attention, so only the live
   keys are gathered (padded to a multiple of 512; pads carry mask 0 and
   cancel on-device exactly like masked keys would).

Device pipeline per core (all matmuls bf16 with fp32 accumulation):
  1. Staging: K.T/V.T/Q.T land in SBUF via a few large DMAs spread over
     the SP/GpSimd/ACT DGE queues so the three streams overlap.
  2. Projections: qT/kT/vT[d2h, s] = W_x.T-slice.T @ X.T-tiles (N=512
     matmuls), + per-partition bias, cast to bf16.  q/k are stored per
     head zero-padded to 128 partitions so score matmuls contract over
     K=128 (fast weight load + near-perfect LDW/MM pipelining).  v is
     PE-transposed per 128-block into v[k, d2h] tiles stored as
     [k, 65]: column 64 holds the key mask value itself - this both
     applies the mask (v and the ones column are zeroed for dead keys)
     and yields the softmax denominator as row 64 of the MM2 output.
  3. Attention, per (head, 512-wide q-chunk), in k-rounds of 3 tiles:
     S_T[k, q] = kT-tile.T @ qT (transposed scores - no transposes
     needed anywhere in the attention path), exp on ScalarE straight
     from 3 PSUM banks (softmax scale folded into the activation's
     scale field; no max-subtraction needed since |s/8| <= ~6), P ->
     bf16 SBUF, acc[65, q] += v_aug.T @ P accumulated over all k-tiles.
     The next round's score matmuls are emitted ahead of this round's
     MM2s so the PE always has independent work while ScalarE runs exp.
     Normalization divides by row 64 (denominator) broadcast across
     partitions with a rank-1 f32r matmul (gpsimd partition_broadcast
     misreads nonzero base partitions on HW) + fast reciprocal.
  4. out[s, :] = sum_h [attn_T_h|1].T @ [W_o slice.T|b_o] - the stored
     attn rows feed straight in as the stationary operand (row 64 = 1
     adds the bias once).  Emission is deferred a few steps so its
     LDWEIGHTS never stalls the PE on the normalization chain.
"""

import sys

for _p in ("/opt/trn_rl_repo", "/opt/pypackages"):
    if _p not in sys.path:
        sys.path.append(_p)

import numpy as np
import ml_dtypes

B = 2
S = 4096
D = 512
H = 8
DK = 64
N_CORES = 8

P = 128          # partitions
QC = 512         # q-chunk width
N_SC = S // QC   # 8 q-chunks
N_ST = S // P    # 32 s-tiles
N_DT = D // P    # 4 D-tiles (contraction tiles for projections)

_COMPILED = {}
_LAST_IN_MAPS = None
_LAST_RESULTS = None
_LAST_NKT = None

COMPACT = True


def _build(nkt: int):
    """Build + compile the per-core bass program. nkt = number of
    128-wide key tiles (32 = full, smaller when keys are compacted)."""
    import concourse.bass as bass  # noqa: F401
    from concourse.masks import make_identity
    import concourse.mybir as mybir
    import concourse.tile as tile
    from concourse import bacc

    f32 = mybir.dt.float32
    f32r = mybir.dt.float32r
    bf16 = mybir.dt.bfloat16
    EXP = mybir.ActivationFunctionType.Exp

    nk = nkt * P
    n_kc = nk // QC

    nc = bacc.Bacc("TRN2", target_bir_lowering=False, debug=False,
                   enable_asserts=False)

    QT = nc.dram_tensor("QT", [D, S], bf16, kind="ExternalInput").ap()
    KT = nc.dram_tensor("KT", [D, nk], bf16, kind="ExternalInput").ap()
    VT = nc.dram_tensor("VT", [D, nk], bf16, kind="ExternalInput").ap()
    WQT = nc.dram_tensor("WQT", [D, P], bf16, kind="ExternalInput").ap()
    WKT = nc.dram_tensor("WKT", [D, P], bf16, kind="ExternalInput").ap()
    WVT = nc.dram_tensor("WVT", [D, P], bf16, kind="ExternalInput").ap()
    WOB = nc.dram_tensor("WOB", [2, P, D], bf16, kind="ExternalInput").ap()
    BQ = nc.dram_tensor("BQ", [P, 1], f32, kind="ExternalInput").ap()
    BK = nc.dram_tensor("BK", [P, 1], f32, kind="ExternalInput").ap()
    BV = nc.dram_tensor("BV", [P, 1], f32, kind="ExternalInput").ap()
    MASKF = nc.dram_tensor("MASKF", [P, nkt], f32, kind="ExternalInput").ap()
    OUT = nc.dram_tensor("OUT", [S, D], f32, kind="ExternalOutput").ap()

    with tile.TileContext(nc) as tc:
        with tc.tile_pool(name="persist", bufs=1) as persist:
            # ---- persistent SBUF tensors ----
            # per-head [128, *] tensors zero-padded in the unused 64
            # partitions so MM1 can contract over K=128 (enables FWL and
            # near-perfect LDW/MM pipelining)
            qh_sb = [persist.tile([P, S], bf16, name=f"qh{h}_sb")
                     for h in range(2)]
            kh_sb = [persist.tile([P, nk], bf16, name=f"kh{h}_sb")
                     for h in range(2)]
            v0_sb = persist.tile([P, nkt, 65], bf16)  # head0 [k, kt, v|mask]
            v1_sb = persist.tile([P, nkt, 65], bf16)  # head1
            a0_sb = persist.tile([P, S], bf16)      # head0 [attn_T|1|0, q]
            a1_sb = persist.tile([P, S], bf16)      # head1
            # raw input staging (few large DMAs beat many small ones:
            # each dma_start costs ~0.6-1us of DGE sequencer time)
            kin_sb = persist.tile([P, N_DT, nk], bf16)
            vin_sb = persist.tile([P, N_DT, nk], bf16)
            qin_sb = persist.tile([P, N_DT, S], bf16)
            wq_sb = persist.tile([P, N_DT, P], bf16)
            wk_sb = persist.tile([P, N_DT, P], bf16)
            wv_sb = persist.tile([P, N_DT, P], bf16)
            wo0_sb = persist.tile([P, D], bf16)
            wo1_sb = persist.tile([P, D], bf16)
            bq_sb = persist.tile([P, 1], f32)
            bk_sb = persist.tile([P, 1], f32)
            bv_sb = persist.tile([P, 1], f32)
            ident_sb = persist.tile([P, P], bf16)
            ones64_sb = persist.tile([65, DK], f32r)
            ones64_f = persist.tile([65, DK], f32)
            mask_sb = persist.tile([P, nkt], f32)

            for t in (*qh_sb, *kh_sb):
                nc.vector.memset(t, 0.0)
            nc.vector.memset(a0_sb, 0.0)
            nc.vector.memset(a1_sb, 0.0)
            make_identity(nc, ident_sb)
            nc.vector.memset(ones64_f, 1.0)
            nc.vector.tensor_copy(out=ones64_sb, in_=ones64_f)

            # DMA queue assignment: SP carries wk + K (the first thing
            # attention needs), GpSimd carries V in parallel, ACT carries
            # the small setup tensors + Q (whose first chunk is split off
            # so q-chunk 0 is projectable within a few us).
            half = (n_kc // 2) * QC
            for dt in range(N_DT):
                dsl = slice(dt * P, (dt + 1) * P)
                nc.sync.dma_start(out=wk_sb[:, dt, :], in_=WKT[dsl, :])
            for dt in range(N_DT):
                dsl = slice(dt * P, (dt + 1) * P)
                nc.sync.dma_start(out=kin_sb[:, dt, 0:half],
                                  in_=KT[dsl, 0:half])
            for dt in range(N_DT):
                dsl = slice(dt * P, (dt + 1) * P)
                nc.gpsimd.dma_start(out=vin_sb[:, dt, 0:half],
                                    in_=VT[dsl, 0:half])
            for dt in range(N_DT):
                dsl = slice(dt * P, (dt + 1) * P)
                nc.gpsimd.dma_start(out=kin_sb[:, dt, half:nk],
                                    in_=KT[dsl, half:nk])
            for dt in range(N_DT):
                dsl = slice(dt * P, (dt + 1) * P)
                nc.sync.dma_start(out=vin_sb[:, dt, half:nk],
                                  in_=VT[dsl, half:nk])
            for dt in range(N_DT):
                dsl = slice(dt * P, (dt + 1) * P)
                nc.scalar.dma_start(out=wv_sb[:, dt, :], in_=WVT[dsl, :])
                nc.scalar.dma_start(out=wq_sb[:, dt, :], in_=WQT[dsl, :])
            nc.scalar.dma_start(out=bk_sb, in_=BK)
            nc.scalar.dma_start(out=bv_sb, in_=BV)
            nc.scalar.dma_start(out=bq_sb, in_=BQ)
            nc.scalar.dma_start(out=mask_sb, in_=MASKF)
            for dt in range(N_DT):
                dsl = slice(dt * P, (dt + 1) * P)
                nc.scalar.dma_start(out=qin_sb[:, dt, 0:QC],
                                    in_=QT[dsl, 0:QC])
            nc.scalar.dma_start(out=wo0_sb, in_=WOB[0])
            nc.scalar.dma_start(out=wo1_sb, in_=WOB[1])
            for dt in range(N_DT):
                dsl = slice(dt * P, (dt + 1) * P)
                nc.scalar.dma_start(out=qin_sb[:, dt, QC:4 * QC],
                                    in_=QT[dsl, QC:4 * QC])
            for dt in range(N_DT):
                dsl = slice(dt * P, (dt + 1) * P)
                eng = nc.sync if dt < 2 else nc.gpsimd
                eng.dma_start(out=qin_sb[:, dt, 4 * QC:S],
                              in_=QT[dsl, 4 * QC:S])
            # column 64 of every v tile = mask value (denominator source)
            nc.vector.tensor_copy(out=v0_sb[:, :, 64], in_=mask_sb)
            nc.vector.tensor_copy(out=v1_sb[:, :, 64], in_=mask_sb)

            # ================= phase P: projections =================
            pin = tc.alloc_tile_pool(name="pin", bufs=2)
            with tc.tile_pool(name="pps", bufs=2, space="PSUM") as pps:

                def qkproj(xin_sb, w_sb, b_sb, x_out, sc, pool=None,
                           tag="qk"):
                    ssl = slice(sc * QC, (sc + 1) * QC)
                    ps = (pool or pps).tile([P, QC], f32, tag=tag,
                                            name="ps", bufs=2)
                    for dt in range(N_DT):
                        nc.tensor.matmul(ps, lhsT=w_sb[:, dt, :],
                                         rhs=xin_sb[:, dt, ssl],
                                         start=(dt == 0),
                                         stop=(dt == N_DT - 1))
                    nc.vector.tensor_scalar_add(
                        out=x_out[0][0:DK, ssl], in0=ps[0:DK, :],
                        scalar1=b_sb[0:DK, :])
                    nc.vector.tensor_scalar_add(
                        out=x_out[1][DK:P, ssl], in0=ps[DK:P, :],
                        scalar1=b_sb[DK:P, :])

                def vproj(vc):
                    # vT[d2h, chunk] (N=512 matmuls), then one PE
                    # transpose per 128-block gives v[k, d2h] for both
                    # heads at once
                    ssl = slice(vc * QC, (vc + 1) * QC)
                    ps = pps.tile([P, QC], f32, tag="qk", name="vps",
                                  bufs=2)
                    for dt in range(N_DT):
                        nc.tensor.matmul(ps, lhsT=wv_sb[:, dt, :],
                                         rhs=vin_sb[:, dt, ssl],
                                         start=(dt == 0),
                                         stop=(dt == N_DT - 1))
                    vt_sb = pin.tile([P, QC], bf16, tag="vt", bufs=2,
                                     name="vt_sb")
                    nc.vector.tensor_scalar_add(out=vt_sb, in0=ps,
                                                scalar1=bv_sb)
                    for st4 in range(QC // P):
                        st = vc * (QC // P) + st4
                        tp = pps.tile([P, P], bf16, tag="v", name="tp",
                                      bufs=2)
                        nc.tensor.transpose(
                            tp, vt_sb[:, st4 * P:(st4 + 1) * P], ident_sb)
                        msl = mask_sb[:, st:st + 1]
                        nc.vector.tensor_scalar_mul(
                            out=v0_sb[:, st, 0:DK], in0=tp[:, 0:DK],
                            scalar1=msl)
                        nc.vector.tensor_scalar_mul(
                            out=v1_sb[:, st, 0:DK], in0=tp[:, DK:P],
                            scalar1=msl)

                for sc in range(n_kc):
                    qkproj(kin_sb, wk_sb, bk_sb, kh_sb, sc)
                for vc in range(n_kc):
                    vproj(vc)
                qkproj(qin_sb, wq_sb, bq_sb, qh_sb, 0)
                _qkproj = qkproj

            # ================= phase A: attention =================
            # k-tile rounds of <=3 (exp reads 3 PSUM banks at once)
            rounds = []
            ki = 0
            while ki < nkt:
                n = min(3, nkt - ki)
                rounds.append((ki, n))
                ki += n

            with tc.tile_pool(name="aps", bufs=1, space="PSUM") as aps, \
                 tc.tile_pool(name="asb", bufs=1) as asb:
                v_sbs = (v0_sb, v1_sb)
                a_sbs = (a0_sb, a1_sb)
                steps = [(h, sc, k0, nk_r)
                         for h in (0, 1)
                         for sc in range(N_SC)
                         for (k0, nk_r) in rounds]

                def emit_mm1(step):
                    h, sc, k0, nk_r = step
                    s_ps = aps.tile([P, 3 * QC], f32, tag="mm1", bufs=2,
                                    name="s_ps")
                    for j in range(nk_r):
                        kt = k0 + j
                        nc.tensor.matmul(
                            s_ps[:, j * QC:(j + 1) * QC],
                            lhsT=kh_sb[h][:, kt * P:(kt + 1) * P],
                            rhs=qh_sb[h][:, sc * QC:(sc + 1) * QC],
                            start=True, stop=True)
                    return s_ps

                def emit_outproj(sc):
                    for st4 in range(QC // P):
                        st = sc * (QC // P) + st4
                        tsl = slice(st * P, (st + 1) * P)
                        po = aps.tile([P, D], f32, tag="acc", bufs=2,
                                      name="po")
                        nc.tensor.matmul(po, lhsT=a0_sb[:, tsl],
                                         rhs=wo0_sb, start=True, stop=False)
                        nc.tensor.matmul(po, lhsT=a1_sb[:, tsl],
                                         rhs=wo1_sb, start=False, stop=True)
                        osb = asb.tile([P, D], f32, tag="osb", bufs=3)
                        nc.vector.tensor_copy(out=osb, in_=po)
                        nc.sync.dma_start(out=OUT[tsl, :], in_=osb)

                s_ps_next = emit_mm1(steps[0])
                acc = None
                pending = []  # (sc, step index when ready)
                for i, step in enumerate(steps):
                    h, sc, k0, nk_r = step
                    ssl = slice(sc * QC, (sc + 1) * QC)
                    v_sb, a_sb = v_sbs[h], a_sbs[h]
                    if h == 0 and k0 == 0 and sc + 1 < N_SC:
                        # project the next q-chunk while this one runs
                        _qkproj(qin_sb, wq_sb, bq_sb, qh_sb, sc + 1,
                                pool=aps, tag="acc")
                    s_ps = s_ps_next
                    p_sb = asb.tile([P, 3 * QC], bf16, tag="p", bufs=3)
                    nc.scalar.activation(
                        p_sb[:, :nk_r * QC], s_ps[:, :nk_r * QC],
                        EXP, bias=0.0, scale=0.125)
                    if i + 1 < len(steps):
                        s_ps_next = emit_mm1(steps[i + 1])
                    if k0 == 0:
                        acc = aps.tile([P, QC], f32, tag="acc", bufs=2,
                                       name="acc")
                    for j in range(nk_r):
                        kt = k0 + j
                        nc.tensor.matmul(
                            acc[0:65, :],
                            lhsT=v_sb[:, kt, :],
                            rhs=p_sb[:, j * QC:(j + 1) * QC],
                            start=(kt == 0), stop=(kt == nkt - 1))
                    if k0 + nk_r == nkt:
                        # normalize: attn = acc[0:64] / acc[64].  Broadcast
                        # the denominator row across partitions with a
                        # rank-1 matmul (f32r streams at full rate; gpsimd
                        # partition_broadcast misreads nonzero base
                        # partitions on HW), then fast reciprocal at base
                        # partition 0.
                        den = asb.tile([65, QC], f32r, tag="den", bufs=2)
                        nc.vector.tensor_copy(out=den[64:65, :],
                                              in_=acc[64:65, :])
                        rb_ps = aps.tile([P, QC], f32, tag="acc", bufs=2,
                                         name="rb_ps")
                        nc.tensor.matmul(
                            rb_ps[0:64, :],
                            lhsT=ones64_sb[64:65, :],
                            rhs=den[64:65, :],
                            start=True, stop=True)
                        recb = asb.tile([64, QC], f32, tag="recb", bufs=2)
                        nc.vector.reciprocal_approx_fast(out=recb,
                                                         in_=rb_ps[0:64, :])
                        nc.vector.tensor_mul(
                            out=a_sb[0:64, ssl], in0=acc[0:64, :],
                            in1=recb)
                        nc.vector.memset(a_sb[64:65, ssl], 1.0)
                        if h == 1:
                            pending.append((sc, i))
                    while pending and i - pending[0][1] >= 4:
                        emit_outproj(pending.pop(0)[0])
                for sc, _ in pending:
                    emit_outproj(sc)
            pin.release()

    nc.compile()
    return nc


def _get_compiled(nkt: int):
    if nkt not in _COMPILED:
        _COMPILED[nkt] = _build(nkt)
    return _COMPILED[nkt]


def kernel(Q, K, V, mask, W_q, b_q, W_k, b_k, W_v, b_v, W_o, b_o):
    from concourse import bass_utils

    bf16 = ml_dtypes.bfloat16
    mask = np.asarray(mask)
    if COMPACT:
        # keys with mask==0 contribute nothing to attention: gather only
        # the live keys (padded to a multiple of 512; pads carry mask 0
        # and are cancelled on-device exactly like masked keys)
        idxs = [np.flatnonzero(mask[b]) for b in range(B)]
        nkt = max(1, -(-max(len(ix) for ix in idxs) // P))
        nkt = min(-(-nkt // 4) * 4, N_ST)  # multiple of 4: 512-wide chunks
    else:
        idxs = None
        nkt = N_ST
    nk = nkt * P

    Q = np.asarray(Q, np.float32)
    K = np.asarray(K, np.float32)
    V = np.asarray(V, np.float32)
    W_q = np.asarray(W_q, np.float32)
    W_k = np.asarray(W_k, np.float32)
    W_v = np.asarray(W_v, np.float32)
    W_o = np.asarray(W_o, np.float32)
    b_q = np.asarray(b_q, np.float32)
    b_k = np.asarray(b_k, np.float32)
    b_v = np.asarray(b_v, np.float32)
    b_o = np.asarray(b_o, np.float32)

    nc = _get_compiled(nkt)

    in_maps = []
    for core in range(N_CORES):
        b = core // 4
        hp = core % 4
        rsl = slice(hp * P, (hp + 1) * P)   # rows of W_q/W_k/W_v, 2 heads
        h0, h1 = 2 * hp, 2 * hp + 1

        qt = np.ascontiguousarray(Q[b].T).astype(bf16)
        if COMPACT:
            ix = idxs[b]
            nkeep = len(ix)
            ixp = np.zeros(nk, np.int64)
            ixp[:nkeep] = ix[:nk]
            kt = np.ascontiguousarray(K[b][ixp, :].T).astype(bf16)
            vt = np.ascontiguousarray(V[b][ixp, :].T).astype(bf16)
            mvec = (np.arange(nk) < min(nkeep, nk)).astype(np.float32)
        else:
            kt = np.ascontiguousarray(K[b].T).astype(bf16)
            vt = np.ascontiguousarray(V[b].T).astype(bf16)
            mvec = mask[b].astype(np.float32)

        wob = np.zeros((2, P, D), np.float32)
        if hp == 0:
            wob[0, 64, :] = b_o
        wob[0, 0:64, :] = W_o[:, h0 * DK:(h0 + 1) * DK].T
        wob[1, 0:64, :] = W_o[:, h1 * DK:(h1 + 1) * DK].T

        maskf = np.ascontiguousarray(mvec.reshape(nkt, P).T)

        in_maps.append({
            "QT": qt,
            "KT": kt,
            "VT": vt,
            "WQT": np.ascontiguousarray(W_q[rsl, :].T).astype(bf16),
            "WKT": np.ascontiguousarray(W_k[rsl, :].T).astype(bf16),
            "WVT": np.ascontiguousarray(W_v[rsl, :].T).astype(bf16),
            "WOB": wob.astype(bf16),
            "BQ": np.ascontiguousarray(b_q[rsl].reshape(P, 1)),
            "BK": np.ascontiguousarray(b_k[rsl].reshape(P, 1)),
            "BV": np.ascontiguousarray(b_v[rsl].reshape(P, 1)),
            "MASKF": maskf,
        })

    global _LAST_IN_MAPS, _LAST_RESULTS, _LAST_NKT
    _LAST_IN_MAPS = in_maps
    _LAST_NKT = nkt

    res = bass_utils.run_bass_kernel_spmd(
        nc, in_maps, core_ids=list(range(N_CORES)))

    _LAST_RESULTS = res.results

    out = np.zeros((B, S, D), np.float32)
    for core in range(N_CORES):
        out[core // 4] += np.asarray(res.results[core]["OUT"], np.float32)
    return out
